# revision 2
# baseline (speedup 1.0000x reference)
"""ChebyshevGCN (K=3) on 8 TRN2 NeuronCores.

Strategy (dst-sharded SpMM via one-hot matmuls, 4-way parallel SWDGE):
  - Nodes dst-sharded across 8 cores (12544 padded rows each); small weights
    replicated. Tables g1 = dis*x and g2 = -dis^2*S (fp16) are AllGathered so
    every core gathers feature rows locally (the "halo exchange").
  - The table is split in 4 quarters (int16 gather indices). Edges are grouped
    (dst-window major, quarter minor); each quarter is an independent gather
    stream pinned to its own SWDGE queue, so up to 4 Q7 cpu-pairs generate
    descriptors concurrently (ucode: cpu_id/2 == queue_num).
  - Per dst window (128 rows), all chunks from the 4 streams accumulate into a
    single PSUM tile; the scatter one-hot oh[e, dstoff] = w_e for the whole
    window is built in 2 batched DVE ops (broadcast-AP is_equal + mult vs an
    iota tile), then drained via Scalar-engine activations directly from PSUM.
  - Chebyshev algebra: out = x@(W0-W2) + Tx1@W1 + (-2 dis*S2)@W2 with
    Tx1 = -dis*S1, so Tx2 is never materialized.
  - Dense epilogue in filter-major form: psum = W'^T @ hT tiles (hT via fp16
    DMA-transpose), relu(+b_cheb) on ACT, then a [filt]x[filt,1] matmul with
    W_lin. Degree/normalization (deg, dis=rsqrt(deg)) computed on device.
"""
import sys
import numpy as np

if "/opt/trn_rl_repo" not in sys.path:
    sys.path.insert(0, "/opt/trn_rl_repo")

import concourse.bass as bass  # noqa: F401
import concourse.mybir as mybir
import concourse.tile as tile
from concourse import bacc, bass_utils

F = 128
GCH = 16          # chunks (of 128 edges) per dma_gather call
DEPTH = 3         # gather-call lookahead per stream
TRACE = [False]   # test.py flips this to get exec_time_ns
LAST_EXEC_NS = [None]


def _ceil(a, b):
    return (a + b - 1) // b


def _plan(x, edge_index, edge_weight, n_cores=8):
    N = x.shape[0]
    S_LOG = _ceil(N, n_cores)
    SHARD = _ceil(S_LOG, 128) * 128
    NTAB = n_cores * SHARD
    QT = NTAB // 4
    assert QT <= 32768
    NW = SHARD // 128

    src = np.asarray(edge_index[0], dtype=np.int64)
    dst = np.asarray(edge_index[1], dtype=np.int64)
    w = np.asarray(edge_weight, dtype=np.float32)

    owner = dst // S_LOG
    dl = dst - owner * S_LOG
    srow = (src // S_LOG) * SHARD + (src % S_LOG)
    q_of = srow // QT
    qidx = (srow % QT).astype(np.int16)
    win = dl // 128
    doff = (dl % 128).astype(np.float32)

    # per-core (w, q) run counts -> shared K[q][w] (chunks per run)
    per_core = []
    cnts = np.zeros((n_cores, NW * 4), np.int64)
    for c in range(n_cores):
        sel = np.nonzero(owner == c)[0]
        qc, wc = q_of[sel], win[sel]
        order = np.lexsort((qc, wc))          # w major, q minor
        sel = sel[order]
        run = win[sel] * 4 + q_of[sel]
        cnts[c] = np.bincount(run, minlength=NW * 4)
        per_core.append(sel)
    Kwq = _ceil(cnts.max(axis=0), 128).reshape(NW, 4)
    Kwq = np.maximum(Kwq, 1)
    K = Kwq.T.copy()                          # [4, NW]
    TOTCH = int(K.sum())
    CW = Kwq.sum(axis=1)                      # chunks per window
    CWMAX = int(CW.max())
    gb = np.concatenate([[0], np.cumsum(CW)])[:-1]        # global chunk base/window
    qoff_w = np.concatenate([np.zeros((NW, 1), np.int64),
                             np.cumsum(Kwq, axis=1)[:, :3]], axis=1)  # [NW, 4]
    sb = np.concatenate([np.zeros((4, 1), np.int64),
                         np.cumsum(K, axis=1)[:, :-1]], axis=1)       # stream pos base
    CQ = K.sum(axis=1)                        # chunks per stream
    NCALLS = [int(_ceil(int(CQ[q]), GCH)) for q in range(4)]
    MAXCALLS = max(NCALLS)

    # gid of chunk (w, q, k) = gb[w] + qoff_w[w, q] + k
    # stream pos of chunk (w, q, k) = sb[q, w] + k

    # out-degree padding for deg reduce
    deg_cnt = np.bincount(src, minlength=N)
    PAD = max(8, _ceil(int(deg_cnt.max()), 8) * 8)

    in_maps = []
    for c in range(n_cores):
        sel = per_core[c]
        runs = win[sel] * 4 + q_of[sel]
        starts = np.concatenate([[0], np.cumsum(cnts[c])])[:-1]
        rank = np.arange(len(sel)) - starts[runs]
        c_loc = rank // 128
        lane = rank % 128
        ws_, qs_ = win[sel], q_of[sel]
        gid = gb[ws_] + qoff_w[ws_, qs_] + c_loc
        spos = sb[qs_, ws_] + c_loc

        dsto_s = np.full((128, TOTCH), 999.0, np.float16)
        ws_s = np.zeros((128, TOTCH), np.float16)
        dsto_s[lane, gid] = doff[sel]
        ws_s[lane, gid] = w[sel]

        idxs = np.zeros((4, MAXCALLS, 128, GCH * 8), np.int16)
        for q in range(4):
            m = qs_ == q
            qarr = np.zeros(int(CQ[q]) * 128, np.int16)
            qarr[spos[m] * 128 + lane[m]] = qidx[sel[m]]
            for i in range(NCALLS[q]):
                nch = min(GCH, int(CQ[q]) - i * GCH)
                ids = qarr[i * GCH * 128:(i * GCH + nch) * 128]
                wrap = ids.reshape(nch * 8, 16).T             # [16, nch*8]
                idxs[q, i, :, :nch * 8] = np.tile(wrap, (8, 1))

        # w_pad for deg (out-edges of own shard nodes)
        sel2 = np.nonzero(src // S_LOG == c)[0]
        loc = (src[sel2] - c * S_LOG).astype(np.int64)
        o2 = np.argsort(loc, kind="stable")
        sel2, loc = sel2[o2], loc[o2]
        c2 = np.bincount(loc, minlength=S_LOG)
        st2 = np.concatenate([[0], np.cumsum(c2)])[:-1]
        rk2 = np.arange(len(sel2)) - st2[loc]
        wpad = np.zeros((NW, 128, PAD), np.float32)
        wpad[loc // 128, loc % 128, rk2] = w[sel2]
        xs = np.zeros((SHARD, F), np.float32)
        n0, n1 = c * S_LOG, min((c + 1) * S_LOG, N)
        xs[: n1 - n0] = np.asarray(x[n0:n1], np.float32)
        in_maps.append({
            "x32": xs, "x16": xs.astype(np.float16), "wpad": wpad,
            "dsto": dsto_s, "wsc": ws_s, "idxs": idxs,
        })
    shape = dict(N=N, S_LOG=S_LOG, SHARD=SHARD, NTAB=NTAB, QT=QT, NW=NW,
                 PAD=PAD, TOTCH=TOTCH, K=K, CW=CW, CWMAX=CWMAX, gb=gb,
                 qoff_w=qoff_w, sb=sb, CQ=CQ, NCALLS=NCALLS,
                 MAXCALLS=MAXCALLS, n_cores=n_cores)
    return shape, in_maps


def _build(p, b_lin_val):
    n_cores, SHARD, NTAB, QT, NW, PAD, TOTCH = (
        p["n_cores"], p["SHARD"], p["NTAB"], p["QT"], p["NW"], p["PAD"],
        p["TOTCH"])
    K, CW, CWMAX, gb, qoff_w, CQ, NCALLS, MAXCALLS = (
        p["K"], p["CW"], p["CWMAX"], p["gb"], p["qoff_w"], p["CQ"],
        p["NCALLS"], p["MAXCALLS"])
    f32, f16, i16, i32 = (mybir.dt.float32, mybir.dt.float16,
                          mybir.dt.int16, mybir.dt.int32)
    Alu, Act = mybir.AluOpType, mybir.ActivationFunctionType

    nc = bacc.Bacc("TRN2", target_bir_lowering=False, debug=False,
                   num_devices=n_cores, num_swdge_queues=4)
    x32 = nc.dram_tensor("x32", [SHARD, F], f32, kind="ExternalInput")
    x16 = nc.dram_tensor("x16", [SHARD, F], f16, kind="ExternalInput")
    wpad = nc.dram_tensor("wpad", [NW, 128, PAD], f32, kind="ExternalInput")
    dsto = nc.dram_tensor("dsto", [128, TOTCH], f16, kind="ExternalInput")
    wsc = nc.dram_tensor("wsc", [128, TOTCH], f16, kind="ExternalInput")
    idxs = nc.dram_tensor("idxs", [4, MAXCALLS, 128, GCH * 8], i16,
                          kind="ExternalInput")
    wch = nc.dram_tensor("wch", [3, 128, 128], f32, kind="ExternalInput")
    bch = nc.dram_tensor("bch", [128, 1], f32, kind="ExternalInput")
    wlin = nc.dram_tensor("wlin", [128, 1], f32, kind="ExternalInput")
    out = nc.dram_tensor("out", [SHARD, 1], f32, kind="ExternalOutput")

    ag1_in = nc.dram_tensor("ag1_in", [SHARD, F], f16, kind="Internal")
    g1_full = nc.dram_tensor("g1_full", [NTAB, F], f16, kind="Internal",
                             addr_space="Shared")
    ag2_in = nc.dram_tensor("ag2_in", [SHARD, F], f16, kind="Internal")
    g2_full = nc.dram_tensor("g2_full", [NTAB, F], f16, kind="Internal",
                             addr_space="Shared")
    tx1s = nc.dram_tensor("tx1s", [SHARD, F], f16, kind="Internal")
    s2s = nc.dram_tensor("s2s", [SHARD, F], f16, kind="Internal")
    rg = [list(range(n_cores))]

    with tile.TileContext(nc) as tc:
        with tc.tile_pool(name="pp", bufs=1) as pp, \
             tc.tile_pool(name="sp", bufs=3) as sp, \
             tc.tile_pool(name="gst", bufs=DEPTH) as gp, \
             tc.tile_pool(name="ip", bufs=DEPTH + 1) as ipool, \
             tc.tile_pool(name="oh", bufs=2) as ohp, \
             tc.tile_pool(name="psA", bufs=2, space="PSUM") as psA, \
             tc.tile_pool(name="psB", bufs=2, space="PSUM") as psB, \
             tc.tile_pool(name="psC", bufs=2, space="PSUM") as psC:

            # ---- prep: scalars, weights, iota -------------------------------
            dsto_t = pp.tile([128, TOTCH], f16)
            nc.sync.dma_start(dsto_t[:], dsto[:, :])
            wsc_t = pp.tile([128, TOTCH], f16)
            nc.sync.dma_start(wsc_t[:], wsc[:, :])
            iota_i = pp.tile([128, 128], i32)
            nc.gpsimd.iota(iota_i[:], pattern=[[1, 128]], base=0,
                           channel_multiplier=0)
            iota_f = pp.tile([128, 128], f32)
            nc.vector.tensor_copy(iota_f[:], iota_i[:])
            w0t = pp.tile([128, 128], f32)
            w2t = pp.tile([128, 128], f32)
            nc.sync.dma_start(w0t[:], wch[0, :, :])
            nc.sync.dma_start(w2t[:], wch[2, :, :])
            w02f = pp.tile([128, 128], f16)
            nc.vector.tensor_tensor(out=w02f[:], in0=w0t[:], in1=w2t[:],
                                    op=Alu.subtract)
            w1f = pp.tile([128, 128], f16)
            w1t = sp.tile([128, 128], f32, tag="wtmp")
            nc.sync.dma_start(w1t[:], wch[1, :, :])
            nc.vector.tensor_copy(w1f[:], w1t[:])
            w2f = pp.tile([128, 128], f16)
            nc.vector.tensor_copy(w2f[:], w2t[:])
            wlt = pp.tile([128, 1], f32)
            nc.sync.dma_start(wlt[:], wlin[:, :])
            wlf = pp.tile([128, 1], f16)
            nc.vector.tensor_copy(wlf[:], wlt[:])
            bcht = pp.tile([128, 1], f32)
            nc.sync.dma_start(bcht[:], bch[:, :])

            # ---- deg / dis --------------------------------------------------
            deg = pp.tile([128, NW], f32)
            for t in range(NW):
                wt = sp.tile([128, PAD], f32, tag="wdeg")
                nc.sync.dma_start(wt[:], wpad[t, :, :])
                nc.vector.tensor_reduce(deg[:, t:t + 1], wt[:],
                                        axis=mybir.AxisListType.X, op=Alu.add)
            dmx = pp.tile([128, NW], f32)
            nc.vector.tensor_scalar(out=dmx[:], in0=deg[:], scalar1=1e-30,
                                    scalar2=None, op0=Alu.max)
            rec = pp.tile([128, NW], f32)
            nc.vector.reciprocal(rec[:], dmx[:])
            sq = pp.tile([128, NW], f32)
            nc.scalar.activation(sq[:], rec[:], Act.Sqrt)
            msk = pp.tile([128, NW], f32)
            nc.vector.tensor_scalar(out=msk[:], in0=deg[:], scalar1=0.0,
                                    scalar2=None, op0=Alu.is_gt)
            dis = pp.tile([128, NW], f32)
            nc.vector.tensor_tensor(out=dis[:], in0=sq[:], in1=msk[:],
                                    op=Alu.mult)
            mdis = pp.tile([128, NW], f32)
            nc.vector.tensor_scalar(out=mdis[:], in0=dis[:], scalar1=-1.0,
                                    scalar2=None, op0=Alu.mult)
            mdis2 = pp.tile([128, NW], f32)
            nc.vector.tensor_tensor(out=mdis2[:], in0=dis[:], in1=mdis[:],
                                    op=Alu.mult)
            m2x = pp.tile([128, NW], f32)
            nc.vector.tensor_scalar(out=m2x[:], in0=dis[:], scalar1=-2.0,
                                    scalar2=None, op0=Alu.mult)

            # ---- g1 = dis * x -> ag1_in; AllGather --------------------------
            for t in range(NW):
                xt = sp.tile([128, F], f32, tag="xprep")
                nc.sync.dma_start(xt[:], x32[t * 128:(t + 1) * 128, :])
                g1t = sp.tile([128, F], f16, tag="g1prep")
                nc.vector.tensor_scalar(out=g1t[:], in0=xt[:],
                                        scalar1=dis[:, t:t + 1], scalar2=None,
                                        op0=Alu.mult)
                nc.sync.dma_start(ag1_in[t * 128:(t + 1) * 128, :], g1t[:])
            nc.gpsimd.collective_compute(
                "AllGather", Alu.bypass, ins=[ag1_in[:, :]],
                outs=[g1_full[:, :]], replica_groups=rg)

            # ---- one SpMM pass over all edges -------------------------------
            def spmm(table, post):
                tiles = [{} for _ in range(4)]        # call idx -> g tile

                def issue(q, i):
                    if i >= NCALLS[q]:
                        return
                    nch = min(GCH, int(CQ[q]) - i * GCH)
                    it = ipool.tile([128, GCH * 8], i16, tag=f"idx{q}")
                    nc.sync.dma_start(it[:, :nch * 8], idxs[q, i, :, :nch * 8])
                    g = gp.tile([128, GCH * 128], f16, tag=f"g{q}")
                    nc.gpsimd.dma_gather(
                        out_ap=g[:, :nch * 128].rearrange(
                            "p (c f) -> p c f", f=F),
                        in_ap=table[q * QT:(q + 1) * QT, :],
                        idxs_ap=it[:, :nch * 8],
                        num_idxs=nch * 128, num_idxs_reg=nch * 128,
                        elem_size=F, single_packet=False,
                        queue_num=q)
                    tiles[q][i] = g

                for i in range(DEPTH):
                    for q in range(4):
                        issue(q, i)

                for w in range(NW):
                    cw = int(CW[w])
                    base = int(gb[w])
                    # batched one-hot for the whole window: 2 DVE ops
                    m = ohp.tile([128, CWMAX * 128], f16, tag="m")
                    oh = ohp.tile([128, CWMAX * 128], f16, tag="oh")
                    m3 = m[:, :cw * 128].rearrange("p (c j) -> p c j", c=cw)
                    oh3 = oh[:, :cw * 128].rearrange("p (c j) -> p c j", c=cw)
                    iota_b = iota_f[:].unsqueeze(1).broadcast_to([128, cw, 128])
                    d_b = dsto_t[:, base:base + cw].unsqueeze(2).broadcast_to(
                        [128, cw, 128])
                    w_b = wsc_t[:, base:base + cw].unsqueeze(2).broadcast_to(
                        [128, cw, 128])
                    nc.vector.tensor_tensor(out=m3, in0=iota_b, in1=d_b,
                                            op=Alu.is_equal)
                    nc.vector.tensor_tensor(out=oh3, in0=m3, in1=w_b,
                                            op=Alu.mult)

                    ps = psA.tile([128, 128], f32, tag="ps")
                    kg = 0
                    for q in range(4):
                        for k in range(int(K[q][w])):
                            spos = int(p["sb"][q][w]) + k
                            call, slot = divmod(spos, GCH)
                            g = tiles[q][call]
                            cq_ = int(qoff_w[w][q]) + k
                            nc.tensor.matmul(
                                out=ps[:],
                                lhsT=oh[:, cq_ * 128:(cq_ + 1) * 128],
                                rhs=g[:, slot * 128:(slot + 1) * 128],
                                start=(kg == 0), stop=(kg == cw - 1))
                            kg += 1
                            if spos == int(CQ[q]) - 1 or slot == GCH - 1:
                                del tiles[q][call]
                                issue(q, call + DEPTH)
                    post(w, ps)

            def post1(t, ps):
                t1 = sp.tile([128, F], f16, tag="tx1")
                nc.scalar.activation(t1[:], ps[:], Act.Copy,
                                     scale=mdis[:, t:t + 1])
                nc.sync.dma_start(tx1s[t * 128:(t + 1) * 128, :], t1[:])
                g2t = sp.tile([128, F], f16, tag="g2e")
                nc.scalar.activation(g2t[:], ps[:], Act.Copy,
                                     scale=mdis2[:, t:t + 1])
                nc.sync.dma_start(ag2_in[t * 128:(t + 1) * 128, :], g2t[:])

            def post2(t, ps):
                s2t = sp.tile([128, F], f16, tag="s2e")
                nc.scalar.activation(s2t[:], ps[:], Act.Copy,
                                     scale=m2x[:, t:t + 1])
                nc.sync.dma_start(s2s[t * 128:(t + 1) * 128, :], s2t[:])

            spmm(g1_full, post1)
            nc.gpsimd.collective_compute(
                "AllGather", Alu.bypass, ins=[ag2_in[:, :]],
                outs=[g2_full[:, :]], replica_groups=rg)
            spmm(g2_full, post2)

            # ---- dense epilogue --------------------------------------------
            for t in range(NW):
                sl = slice(t * 128, (t + 1) * 128)
                xT = sp.tile([128, 128], f16, tag="xT")
                nc.sync.dma_start(xT[:], x16[sl, :], transpose=True)
                t1T = sp.tile([128, 128], f16, tag="t1T")
                nc.sync.dma_start(t1T[:], tx1s[sl, :], transpose=True)
                s2T = sp.tile([128, 128], f16, tag="s2T")
                nc.sync.dma_start(s2T[:], s2s[sl, :], transpose=True)
                po = psB.tile([128, 128], f32, tag="po")
                nc.tensor.matmul(out=po[:], lhsT=w02f[:], rhs=xT[:],
                                 start=True, stop=False)
                nc.tensor.matmul(out=po[:], lhsT=w1f[:], rhs=t1T[:],
                                 start=False, stop=False)
                nc.tensor.matmul(out=po[:], lhsT=w2f[:], rhs=s2T[:],
                                 start=False, stop=True)
                rl = sp.tile([128, 128], f16, tag="rl")
                nc.scalar.activation(rl[:], po[:], Act.Relu, bias=bcht[:])
                pf = psC.tile([128, 1], f32, tag="pf")
                nc.tensor.matmul(out=pf[:], lhsT=rl[:], rhs=wlf[:],
                                 start=True, stop=True)
                yt = sp.tile([128, 1], f32, tag="yt")
                nc.vector.tensor_scalar(out=yt[:], in0=pf[:],
                                        scalar1=float(b_lin_val), scalar2=None,
                                        op0=Alu.add)
                nc.sync.dma_start(out[sl, :], yt[:])
    nc.compile()
    return nc


def kernel(x, edge_index, edge_weight, W_cheb, b_cheb, W_lin, b_lin):
    x = np.asarray(x)
    n_cores = 8
    p, in_maps = _plan(x, np.asarray(edge_index), np.asarray(edge_weight),
                       n_cores)
    wch = np.asarray(W_cheb, np.float32)
    bch = np.asarray(b_cheb, np.float32).reshape(128, 1)
    wl = np.asarray(W_lin, np.float32).reshape(128, 1)
    blv = float(np.asarray(b_lin).reshape(-1)[0])
    for m in in_maps:
        m["wch"] = wch
        m["bch"] = bch
        m["wlin"] = wl
    nc = _build(p, blv)
    r = bass_utils.run_bass_kernel_spmd(
        nc, in_maps, core_ids=list(range(n_cores)), trace=TRACE[0])
    LAST_EXEC_NS[0] = r.exec_time_ns
    S_LOG, N = p["S_LOG"], p["N"]
    outs = [np.asarray(r.results[c]["out"])[:min(S_LOG, N - c * S_LOG)]
            for c in range(n_cores)]
    return np.concatenate(outs, axis=0).astype(np.float32)


# revision 10
# speedup vs baseline: 1.3143x; 1.3143x over previous
"""ChebyshevGCN (K=3) on 8 TRN2 NeuronCores.

Strategy (dst-sharded SpMM via one-hot matmuls, 4-way parallel SWDGE):
  - Nodes dst-sharded across 8 cores (12544 padded rows each); small weights
    replicated. Tables g1 = dis*x and g2 = -dis^2*S (fp16) are AllGathered so
    every core gathers feature rows locally (the "halo exchange"). Each table
    is AllGathered in two row-halves (A = local rows [0,6272), B = rest) so
    the first collective overlaps the producing pass.
  - The halo table is split in 4 subtables of 25088 rows (int16 gather
    indices): A-lo, A-hi, B-lo, B-hi. Edges are grouped (dst-window major,
    subtable minor); each subtable is an independent gather stream pinned to
    its own SWDGE queue, so up to 4 Q7 cpu-pairs generate DMA descriptors
    concurrently (ucode dispatch: cpu_id/2 == queue_num).
  - Per dst window (128 rows), all chunks from the 4 streams accumulate into
    one PSUM tile; the scatter one-hot oh[e, dstoff] = w_e for the whole
    window is built in 2 batched all-fp16 DVE ops (broadcast-AP is_equal +
    mult vs an iota tile), then drained via Scalar-engine activations
    directly from PSUM.
  - Chebyshev algebra: out = x@(W0-W2) + Tx1@W1 + (-2 dis*S2)@W2 with
    Tx1 = -dis*S1, so Tx2 is never materialized.
  - The dense epilogue is folded into the passes: xT tiles are DMA-transposed
    during pass 1, Tx1^T is PE-transposed into SBUF as pass 1 drains, and in
    pass 2 each window computes psum = W'^T @ hT, relu(+b_cheb) on ACT, and
    the final [filt]x[filt,1] matmul with W_lin right after its SpMM chunk.
    Degree/normalization (deg, dis=rsqrt(deg)) computed on device.
"""
import sys
import numpy as np

if "/opt/trn_rl_repo" not in sys.path:
    sys.path.insert(0, "/opt/trn_rl_repo")

import concourse.bass as bass  # noqa: F401
import concourse.mybir as mybir
import concourse.tile as tile
from concourse import bacc, bass_utils

F = 128
GCH = 16          # chunks (of 128 edges) per dma_gather call
DEPTH = 4         # gather-call lookahead per stream
PB = 7            # windows batched per prep op
TRACE = [False]   # test.py flips this to get exec_time_ns
LAST_EXEC_NS = [None]


def _ceil(a, b):
    return (a + b - 1) // b


def _plan(x, edge_index, edge_weight, n_cores=8):
    N = x.shape[0]
    S_LOG = _ceil(N, n_cores)
    SHARD = _ceil(S_LOG, 128) * 128
    HALF = SHARD // 2
    NTAB = n_cores * SHARD
    QT = NTAB // 4
    assert QT <= 32768
    NW = SHARD // 128

    src = np.asarray(edge_index[0], dtype=np.int64)
    dst = np.asarray(edge_index[1], dtype=np.int64)
    w = np.asarray(edge_weight, dtype=np.float32)

    owner = dst // S_LOG
    dl = dst - owner * S_LOG
    # table rows: halves-major layout (tables A and B, each [8*HALF, F])
    sc = src // S_LOG
    sl = src - sc * S_LOG
    hi = sl >= HALF
    srow = np.where(hi, 4 * QT // 2 + sc * HALF + (sl - HALF), sc * HALF + sl)
    q_of = srow // QT
    qidx = (srow % QT).astype(np.int16)
    win = dl // 128
    doff = (dl % 128).astype(np.float16)

    # per-core (w, q) run counts -> shared K[q][w] (chunks per run)
    per_core = []
    cnts = np.zeros((n_cores, NW * 4), np.int64)
    for c in range(n_cores):
        sel = np.nonzero(owner == c)[0]
        qc, wc = q_of[sel], win[sel]
        order = np.lexsort((qc, wc))          # w major, q minor
        sel = sel[order]
        run = win[sel] * 4 + q_of[sel]
        cnts[c] = np.bincount(run, minlength=NW * 4)
        per_core.append(sel)
    Kwq = _ceil(cnts.max(axis=0), 128).reshape(NW, 4)
    Kwq = np.maximum(Kwq, 1)
    K = Kwq.T.copy()                          # [4, NW]
    TOTCH = int(K.sum())
    CW = Kwq.sum(axis=1)                      # chunks per window
    CWMAX = int(CW.max())
    gb = np.concatenate([[0], np.cumsum(CW)])[:-1]        # global chunk base/window
    qoff_w = np.concatenate([np.zeros((NW, 1), np.int64),
                             np.cumsum(Kwq, axis=1)[:, :3]], axis=1)  # [NW, 4]
    sb = np.concatenate([np.zeros((4, 1), np.int64),
                         np.cumsum(K, axis=1)[:, :-1]], axis=1)       # stream pos base
    CQ = K.sum(axis=1)                        # chunks per stream
    NCALLS = [int(_ceil(int(CQ[q]), GCH)) for q in range(4)]
    MAXCALLS = max(NCALLS)

    # out-degree padding for deg reduce
    deg_cnt = np.bincount(src, minlength=N)
    PAD = max(8, _ceil(int(deg_cnt.max()), 8) * 8)

    in_maps = []
    for c in range(n_cores):
        sel = per_core[c]
        runs = win[sel] * 4 + q_of[sel]
        starts = np.concatenate([[0], np.cumsum(cnts[c])])[:-1]
        rank = np.arange(len(sel)) - starts[runs]
        c_loc = rank // 128
        lane = rank % 128
        ws_, qs_ = win[sel], q_of[sel]
        gid = gb[ws_] + qoff_w[ws_, qs_] + c_loc
        spos = sb[qs_, ws_] + c_loc

        dsto_s = np.full((128, TOTCH), -1.0, np.float16)
        ws_s = np.zeros((128, TOTCH), np.float16)
        dsto_s[lane, gid] = doff[sel]
        ws_s[lane, gid] = w[sel]

        idxs = np.zeros((4, MAXCALLS, 128, GCH * 8), np.int16)
        for q in range(4):
            m = qs_ == q
            qarr = np.zeros(int(CQ[q]) * 128, np.int16)
            qarr[spos[m] * 128 + lane[m]] = qidx[sel[m]]
            for i in range(NCALLS[q]):
                nch = min(GCH, int(CQ[q]) - i * GCH)
                ids = qarr[i * GCH * 128:(i * GCH + nch) * 128]
                wrap = ids.reshape(nch * 8, 16).T             # [16, nch*8]
                idxs[q, i, :, :nch * 8] = np.tile(wrap, (8, 1))

        # w_pad for deg (out-edges of own shard nodes)
        sel2 = np.nonzero(sc == c)[0]
        loc = sl[sel2]
        o2 = np.argsort(loc, kind="stable")
        sel2, loc = sel2[o2], loc[o2]
        c2 = np.bincount(loc, minlength=S_LOG)
        st2 = np.concatenate([[0], np.cumsum(c2)])[:-1]
        rk2 = np.arange(len(sel2)) - st2[loc]
        wpad = np.zeros((NW, 128, PAD), np.float32)
        wpad[loc // 128, loc % 128, rk2] = w[sel2]
        xs = np.zeros((SHARD, F), np.float32)
        n0, n1 = c * S_LOG, min((c + 1) * S_LOG, N)
        xs[: n1 - n0] = np.asarray(x[n0:n1], np.float32)
        in_maps.append({
            "x32": xs, "x16": xs.astype(np.float16), "wpad": wpad,
            "dsto": dsto_s, "wsc": ws_s, "idxs": idxs,
        })
    shape = dict(N=N, S_LOG=S_LOG, SHARD=SHARD, HALF=HALF, NTAB=NTAB, QT=QT,
                 NW=NW, PAD=PAD, TOTCH=TOTCH, K=K, CW=CW, CWMAX=CWMAX, gb=gb,
                 qoff_w=qoff_w, sb=sb, CQ=CQ, NCALLS=NCALLS,
                 MAXCALLS=MAXCALLS, n_cores=n_cores)
    return shape, in_maps


def _build(p, b_lin_val):
    n_cores, SHARD, HALF, QT, NW, PAD, TOTCH = (
        p["n_cores"], p["SHARD"], p["HALF"], p["QT"], p["NW"], p["PAD"],
        p["TOTCH"])
    K, CW, CWMAX, gb, qoff_w, CQ, NCALLS, MAXCALLS = (
        p["K"], p["CW"], p["CWMAX"], p["gb"], p["qoff_w"], p["CQ"],
        p["NCALLS"], p["MAXCALLS"])
    NHALF = n_cores * HALF                    # rows per half-table (2 streams)
    WH = HALF // 128                          # windows per half (49)
    f32, f16, i16, i32 = (mybir.dt.float32, mybir.dt.float16,
                          mybir.dt.int16, mybir.dt.int32)
    Alu, Act = mybir.AluOpType, mybir.ActivationFunctionType

    nc = bacc.Bacc("TRN2", target_bir_lowering=False, debug=False,
                   num_devices=n_cores, num_swdge_queues=4)
    x32 = nc.dram_tensor("x32", [SHARD, F], f32, kind="ExternalInput")
    x16 = nc.dram_tensor("x16", [SHARD, F], f16, kind="ExternalInput")
    wpad = nc.dram_tensor("wpad", [NW, 128, PAD], f32, kind="ExternalInput")
    dsto = nc.dram_tensor("dsto", [128, TOTCH], f16, kind="ExternalInput")
    wsc = nc.dram_tensor("wsc", [128, TOTCH], f16, kind="ExternalInput")
    idxs = nc.dram_tensor("idxs", [4, MAXCALLS, 128, GCH * 8], i16,
                          kind="ExternalInput")
    wch = nc.dram_tensor("wch", [3, 128, 128], f32, kind="ExternalInput")
    bch = nc.dram_tensor("bch", [128, 1], f32, kind="ExternalInput")
    wlin = nc.dram_tensor("wlin", [128, 1], f32, kind="ExternalInput")
    out = nc.dram_tensor("out", [SHARD, 1], f32, kind="ExternalOutput")

    ag1A = nc.dram_tensor("ag1A", [HALF, F], f16, kind="Internal")
    ag1B = nc.dram_tensor("ag1B", [HALF, F], f16, kind="Internal")
    g1A = nc.dram_tensor("g1A", [NHALF, F], f16, kind="Internal",
                         addr_space="Shared")
    g1B = nc.dram_tensor("g1B", [NHALF, F], f16, kind="Internal",
                         addr_space="Shared")
    ag2A = nc.dram_tensor("ag2A", [HALF, F], f16, kind="Internal")
    ag2B = nc.dram_tensor("ag2B", [HALF, F], f16, kind="Internal")
    g2A = nc.dram_tensor("g2A", [NHALF, F], f16, kind="Internal",
                         addr_space="Shared")
    g2B = nc.dram_tensor("g2B", [NHALF, F], f16, kind="Internal",
                         addr_space="Shared")
    rg = [list(range(n_cores))]

    with tile.TileContext(nc) as tc:
        with tc.tile_pool(name="pp", bufs=1) as pp, \
             tc.tile_pool(name="sp", bufs=3) as sp, \
             tc.tile_pool(name="gst", bufs=DEPTH) as gp, \
             tc.tile_pool(name="ip", bufs=DEPTH + 1) as ipool, \
             tc.tile_pool(name="oh", bufs=2) as ohp, \
             tc.tile_pool(name="psA", bufs=2, space="PSUM") as psA, \
             tc.tile_pool(name="psT", bufs=2, space="PSUM") as psT, \
             tc.tile_pool(name="psB", bufs=2, space="PSUM") as psB, \
             tc.tile_pool(name="psC", bufs=2, space="PSUM") as psC:

            # ---- prep: scalars, weights, iota, identity ---------------------
            dsto_t = pp.tile([128, TOTCH], f16)
            nc.sync.dma_start(dsto_t[:], dsto[:, :])
            wsc_t = pp.tile([128, TOTCH], f16)
            nc.sync.dma_start(wsc_t[:], wsc[:, :])
            iota_i = pp.tile([128, 128], i32)
            nc.gpsimd.iota(iota_i[:], pattern=[[1, 128]], base=0,
                           channel_multiplier=0)
            iota_h = pp.tile([128, 128], f16)
            nc.vector.tensor_copy(iota_h[:], iota_i[:])
            pidx_i = pp.tile([128, 1], i32)
            nc.gpsimd.iota(pidx_i[:], pattern=[[1, 1]], base=0,
                           channel_multiplier=1)
            pidx_f = pp.tile([128, 1], f32)
            nc.vector.tensor_copy(pidx_f[:], pidx_i[:])
            ident = pp.tile([128, 128], f16)
            nc.vector.tensor_scalar(out=ident[:], in0=iota_h[:],
                                    scalar1=pidx_f[:], scalar2=None,
                                    op0=Alu.is_equal)
            w0t = pp.tile([128, 128], f32)
            w2t = pp.tile([128, 128], f32)
            nc.sync.dma_start(w0t[:], wch[0, :, :])
            nc.sync.dma_start(w2t[:], wch[2, :, :])
            w02f = pp.tile([128, 128], f16)
            nc.vector.tensor_tensor(out=w02f[:], in0=w0t[:], in1=w2t[:],
                                    op=Alu.subtract)
            w1f = pp.tile([128, 128], f16)
            w1t = sp.tile([128, 128], f32, tag="wtmp")
            nc.sync.dma_start(w1t[:], wch[1, :, :])
            nc.vector.tensor_copy(w1f[:], w1t[:])
            w2f = pp.tile([128, 128], f16)
            nc.vector.tensor_copy(w2f[:], w2t[:])
            wlt = pp.tile([128, 1], f32)
            nc.sync.dma_start(wlt[:], wlin[:, :])
            wlf = pp.tile([128, 1], f16)
            nc.vector.tensor_copy(wlf[:], wlt[:])
            bcht = pp.tile([128, 1], f32)
            nc.sync.dma_start(bcht[:], bch[:, :])

            # ---- deg / dis --------------------------------------------------
            deg = pp.tile([128, NW], f32)
            for t in range(NW):
                wt = sp.tile([128, PAD], f32, tag="wdeg")
                nc.sync.dma_start(wt[:], wpad[t, :, :])
                nc.vector.tensor_reduce(deg[:, t:t + 1], wt[:],
                                        axis=mybir.AxisListType.X, op=Alu.add)
            dmx = pp.tile([128, NW], f32)
            nc.vector.tensor_scalar(out=dmx[:], in0=deg[:], scalar1=1e-30,
                                    scalar2=None, op0=Alu.max)
            rec = pp.tile([128, NW], f32)
            nc.vector.reciprocal(rec[:], dmx[:])
            sq = pp.tile([128, NW], f32)
            nc.scalar.activation(sq[:], rec[:], Act.Sqrt)
            msk = pp.tile([128, NW], f32)
            nc.vector.tensor_scalar(out=msk[:], in0=deg[:], scalar1=0.0,
                                    scalar2=None, op0=Alu.is_gt)
            dis = pp.tile([128, NW], f32)
            nc.vector.tensor_tensor(out=dis[:], in0=sq[:], in1=msk[:],
                                    op=Alu.mult)
            mdis = pp.tile([128, NW], f32)
            nc.vector.tensor_scalar(out=mdis[:], in0=dis[:], scalar1=-1.0,
                                    scalar2=None, op0=Alu.mult)
            mdis2 = pp.tile([128, NW], f32)
            nc.vector.tensor_tensor(out=mdis2[:], in0=dis[:], in1=mdis[:],
                                    op=Alu.mult)
            m2x = pp.tile([128, NW], f32)
            nc.vector.tensor_scalar(out=m2x[:], in0=dis[:], scalar1=-2.0,
                                    scalar2=None, op0=Alu.mult)

            # ---- g1 = dis * x -> ag1A/ag1B; sliced AllGather ----------------
            def g1prep(t, dst_dram, off):
                xt = sp.tile([128, F], f32, tag="xprep")
                nc.sync.dma_start(xt[:], x32[t * 128:(t + 1) * 128, :])
                g1t = sp.tile([128, F], f16, tag="g1prep")
                nc.vector.tensor_scalar(out=g1t[:], in0=xt[:],
                                        scalar1=dis[:, t:t + 1], scalar2=None,
                                        op0=Alu.mult)
                nc.sync.dma_start(
                    dst_dram[(t - off) * 128:(t - off + 1) * 128, :], g1t[:])

            for t in range(WH):
                g1prep(t, ag1A, 0)
            nc.gpsimd.collective_compute(
                "AllGather", Alu.bypass, ins=[ag1A[:, :]],
                outs=[g1A[:, :]], replica_groups=rg)
            for t in range(WH, NW):
                g1prep(t, ag1B, WH)
            nc.gpsimd.collective_compute(
                "AllGather", Alu.bypass, ins=[ag1B[:, :]],
                outs=[g1B[:, :]], replica_groups=rg)

            xT_all = pp.tile([128, NW * 128], f16)
            t1T_all = pp.tile([128, NW * 128], f16)

            # ---- one SpMM pass over all edges -------------------------------
            def spmm(tabA, tabB, post):
                tiles = [{} for _ in range(4)]        # call idx -> g tile
                tabs = [tabA, tabA, tabB, tabB]
                offs = [0, QT, 0, QT]

                def issue(q, i):
                    if i >= NCALLS[q]:
                        return
                    nch = min(GCH, int(CQ[q]) - i * GCH)
                    it = ipool.tile([128, GCH * 8], i16, tag=f"idx{q}")
                    nc.sync.dma_start(it[:, :nch * 8], idxs[q, i, :, :nch * 8])
                    g = gp.tile([128, GCH * 128], f16, tag=f"g{q}")
                    nc.gpsimd.dma_gather(
                        out_ap=g[:, :nch * 128].rearrange(
                            "p (c f) -> p c f", f=F),
                        in_ap=tabs[q][offs[q]:offs[q] + QT, :],
                        idxs_ap=it[:, :nch * 8],
                        num_idxs=nch * 128, num_idxs_reg=nch * 128,
                        elem_size=F, single_packet=False,
                        queue_num=q)
                    tiles[q][i] = g

                for i in range(DEPTH):
                    for q in range(4):
                        issue(q, i)

                for w in range(NW):
                    cw = int(CW[w])
                    base = int(gb[w])
                    # batched one-hot for the whole window: 2 fp16 DVE ops
                    m = ohp.tile([128, CWMAX * 128], f16, tag="m")
                    oh = ohp.tile([128, CWMAX * 128], f16, tag="oh")
                    m3 = m[:, :cw * 128].rearrange("p (c j) -> p c j", c=cw)
                    oh3 = oh[:, :cw * 128].rearrange("p (c j) -> p c j", c=cw)
                    iota_b = iota_h[:].unsqueeze(1).broadcast_to([128, cw, 128])
                    d_b = dsto_t[:, base:base + cw].unsqueeze(2).broadcast_to(
                        [128, cw, 128])
                    w_b = wsc_t[:, base:base + cw].unsqueeze(2).broadcast_to(
                        [128, cw, 128])
                    nc.vector.tensor_tensor(out=m3, in0=iota_b, in1=d_b,
                                            op=Alu.is_equal)
                    nc.vector.tensor_tensor(out=oh3, in0=m3, in1=w_b,
                                            op=Alu.mult)

                    ps = psA.tile([128, 128], f32, tag="ps")
                    kg = 0
                    for q in range(4):
                        for k in range(int(K[q][w])):
                            spos = int(p["sb"][q][w]) + k
                            call, slot = divmod(spos, GCH)
                            g = tiles[q][call]
                            cq_ = int(qoff_w[w][q]) + k
                            nc.tensor.matmul(
                                out=ps[:],
                                lhsT=oh[:, cq_ * 128:(cq_ + 1) * 128],
                                rhs=g[:, slot * 128:(slot + 1) * 128],
                                start=(kg == 0), stop=(kg == cw - 1))
                            kg += 1
                            if spos == int(CQ[q]) - 1 or slot == GCH - 1:
                                del tiles[q][call]
                                issue(q, call + DEPTH)
                    post(w, ps)

            # ---- pass 1 post: Tx1 (transposed, SBUF) + g2 table + xT --------
            def post1(t, ps):
                t1 = sp.tile([128, F], f16, tag="tx1")
                nc.scalar.activation(t1[:], ps[:], Act.Copy,
                                     scale=mdis[:, t:t + 1])
                pt = psT.tile([128, 128], f16, tag="pt")
                nc.tensor.transpose(pt[:], t1[:], ident[:])
                nc.vector.tensor_copy(t1T_all[:, t * 128:(t + 1) * 128], pt[:])
                g2t = sp.tile([128, F], f16, tag="g2e")
                nc.scalar.activation(g2t[:], ps[:], Act.Copy,
                                     scale=mdis2[:, t:t + 1])
                ag2 = ag2A if t < WH else ag2B
                toff = t if t < WH else t - WH
                nc.sync.dma_start(ag2[toff * 128:(toff + 1) * 128, :], g2t[:])
                nc.sync.dma_start(xT_all[:, t * 128:(t + 1) * 128],
                                  x16[t * 128:(t + 1) * 128, :],
                                  transpose=True)
                if t == WH - 1:
                    nc.gpsimd.collective_compute(
                        "AllGather", Alu.bypass, ins=[ag2A[:, :]],
                        outs=[g2A[:, :]], replica_groups=rg)

            # ---- pass 2 post: s2, fused dense epilogue ----------------------
            def post2(t, ps):
                s2t = sp.tile([128, F], f16, tag="s2e")
                nc.scalar.activation(s2t[:], ps[:], Act.Copy,
                                     scale=m2x[:, t:t + 1])
                pt = psT.tile([128, 128], f16, tag="pt")
                nc.tensor.transpose(pt[:], s2t[:], ident[:])
                s2T = sp.tile([128, 128], f16, tag="s2T")
                nc.vector.tensor_copy(s2T[:], pt[:])
                po = psB.tile([128, 128], f32, tag="po")
                nc.tensor.matmul(out=po[:], lhsT=w02f[:],
                                 rhs=xT_all[:, t * 128:(t + 1) * 128],
                                 start=True, stop=False)
                nc.tensor.matmul(out=po[:], lhsT=w1f[:],
                                 rhs=t1T_all[:, t * 128:(t + 1) * 128],
                                 start=False, stop=False)
                nc.tensor.matmul(out=po[:], lhsT=w2f[:], rhs=s2T[:],
                                 start=False, stop=True)
                rl = sp.tile([128, 128], f16, tag="rl")
                nc.scalar.activation(rl[:], po[:], Act.Relu, bias=bcht[:])
                pf = psC.tile([128, 1], f32, tag="pf")
                nc.tensor.matmul(out=pf[:], lhsT=rl[:], rhs=wlf[:],
                                 start=True, stop=True)
                yt = sp.tile([128, 1], f32, tag="yt")
                nc.scalar.activation(yt[:], pf[:], Act.Copy,
                                     bias=float(b_lin_val))
                nc.sync.dma_start(out[t * 128:(t + 1) * 128, :], yt[:])

            spmm(g1A, g1B, post1)
            nc.gpsimd.collective_compute(
                "AllGather", Alu.bypass, ins=[ag2B[:, :]],
                outs=[g2B[:, :]], replica_groups=rg)
            spmm(g2A, g2B, post2)
    nc.compile()
    return nc


def kernel(x, edge_index, edge_weight, W_cheb, b_cheb, W_lin, b_lin):
    x = np.asarray(x)
    n_cores = 8
    p, in_maps = _plan(x, np.asarray(edge_index), np.asarray(edge_weight),
                       n_cores)
    wch = np.asarray(W_cheb, np.float32)
    bch = np.asarray(b_cheb, np.float32).reshape(128, 1)
    wl = np.asarray(W_lin, np.float32).reshape(128, 1)
    blv = float(np.asarray(b_lin).reshape(-1)[0])
    for m in in_maps:
        m["wch"] = wch
        m["bch"] = bch
        m["wlin"] = wl
    nc = _build(p, blv)
    r = bass_utils.run_bass_kernel_spmd(
        nc, in_maps, core_ids=list(range(n_cores)), trace=TRACE[0])
    LAST_EXEC_NS[0] = r.exec_time_ns
    S_LOG, N = p["S_LOG"], p["N"]
    outs = [np.asarray(r.results[c]["out"])[:min(S_LOG, N - c * S_LOG)]
            for c in range(n_cores)]
    return np.concatenate(outs, axis=0).astype(np.float32)
